# revision 10
# baseline (speedup 1.0000x reference)
"""KNN graph kernel (DenseDilatedKnnGraph) for Trainium2, 8 NeuronCores.

Problem: x [2, 192, 8192, 1] fp32 -> edge_index [2, 2, 8192, 9] int32.
reference: L2-normalize x along C, pairwise sq-dists over N, top-9 (k=9,
dilation=1) nearest neighbors (indices), stacked with center indices.

Strategy (candidate-screen + exact host rerank):
  For normalized points, ranking by -dist == ranking by cosine Xn^T Xn.
  The device computes an fp16 SCREENING Gram (2 K=128 matmul passes per
  512-col chunk; B channels zero-padded to K=128 so the PE tile config
  never changes -- a K=64/K=128 alternation drops the PE to its mid
  pstate) and reduces each query row's 8192 columns to a 256-slot
  column-max array via a DVE fp16 tensor_max fold tree (fp16
  tensor_tensor runs at 2x_1p; MAX8/FIND_INDEX8 have no fast mode, so
  their scan area must be small). Top-8 slots of each 64-slot quarter
  -> 32 slots x 32 cols = 1024 candidate columns per row; the host
  re-ranks candidates exactly (f32 screen + f64 top-16 refine) against
  the fp32 normalized points, reproducing the reference top-8.

  Slot geometry: the host PERMUTES moving columns P(c) = (c%16)*512 +
  c//16, so slot s (= permuted col mod 256) covers the 16-contiguous-
  col groups {16s..16s+15, 16(s+256)..16(s+256)+15}. The data is
  diagonally correlated (neighbors cluster at col ~ row), so contiguous
  groups absorb clusters of near neighbors into one slot instead of
  crowding many distinct slots. Top-8-per-quarter is safe: a true top-8
  item's slot can have at most 7 slots with larger max anywhere (each
  must contain a better item), so only fp16 ties at the exact 8/9
  quarter boundary can drop it -- requiring all 7 above-slots in one
  quarter, P ~ (1/4)^7 per row, ~0 expected over 16k rows.

  The self column (cos ~ 1) is knocked out with -20 via one constant
  [128,16,8] mask (its permuted position is row-dependent) before
  folding; the host prepends the self index (reference rank-1 neighbor
  is always self).

Sharding: 8 cores = 2 batches x 4 query-row-blocks of 2048. Each core
gets the full batch slice with columns ROTATED so its query block sits
at rotated columns 0..2047. Host maps indices back via the offset.

Per row-tile (128 rows x 8192 cols) engine budget, measured rates:
  PE   32 matmuls at ~248ns cadence             = 7.9us   <- bound
  ACT  4 drains [128,2048] PSUM->SBUF fp16      = 7.9us   <- bound
  DVE  folds 4.7 + mask 0.2 + scans 1.8         = 6.7us
Startup ~10us (6us NEFF preamble + input DMA, overlapped with PE
pstate warmup), tail ~5us (last tile's fold+scan chain).
"""

import numpy as np

B = 2
C = 192
N = 8192
NCORES = 8
RBLK = N // 4  # 2048 query rows per core
NT = RBLK // 128  # 16 row tiles per core
NSLOT = 256
NEG = -20.0

_cache = {}


def _self_mask():
    """m[p, i, d] = NEG iff the self column of query row p lands at
    permuted column i*512 + 8t + d (t enters via the AP column offset;
    the mask data is t-independent)."""
    m = np.zeros((128, 16, 8), np.float16)
    for p in range(128):
        m[p, p % 16, p // 16] = np.float16(NEG)
    return m.reshape(128, 128)


def _build_nc(nt=NT):
    import concourse.bacc as bacc
    import concourse.mybir as mybir
    from concourse.bass import ts
    from concourse.tile import TileContext

    f32 = mybir.dt.float32
    f16 = mybir.dt.float16
    u16 = mybir.dt.uint16

    nc = bacc.Bacc("TRN2")

    # moving columns: rotated + permuted; query columns: rotated only
    xm = nc.dram_tensor("xm", [C, N], f16, kind="ExternalInput")
    xq = nc.dram_tensor("xq", [C, RBLK], f16, kind="ExternalInput")
    idx_out = nc.dram_tensor("idx8", [RBLK, 32], u16, kind="ExternalOutput")
    val_out = nc.dram_tensor("val8", [RBLK, 32], f16, kind="ExternalOutput")

    mask_d = nc.inline_tensor(_self_mask(), name="selfmask")

    with TileContext(nc) as tc:
        with (
            tc.tile_pool(name="consts", bufs=1) as cpool,
            tc.tile_pool(name="xpool", bufs=1) as xpool,
            tc.tile_pool(name="spool", bufs=2) as spool,
            tc.tile_pool(name="fpool", bufs=3) as fpool,
            tc.tile_pool(name="vpool", bufs=3) as vpool,
            tc.tile_pool(name="gpsum", bufs=2, space="PSUM") as gpsum,
        ):
            # fp16 normalized points (host-prepared). A: channels 0..127;
            # B: channels 128..191 in rows 0..63, rows 64..127 zeroed.
            # DMA issues are ~620ns each, serialized on the Sync engine:
            # order = query block first (unblocks the PE warmup), then
            # moving blocks (first real tile depends only on block 0),
            # mask last.
            hqA = xpool.tile([128, RBLK], f16)
            hqB = xpool.tile([128, RBLK], f16)
            nc.gpsimd.memset(hqB[64:128, :], 0.0)
            nc.sync.dma_start(hqA, xq[0:128, :])
            nc.sync.dma_start(hqB[0:64, :], xq[128:192, :])
            hmA, hmB = [], []
            for q in range(4):
                hma = xpool.tile([128, 2048], f16, tag=f"hmA{q}")
                hmb = xpool.tile([128, 2048], f16, tag=f"hmB{q}")
                nc.gpsimd.memset(hmb[64:128, :], 0.0)
                qsl = ts(q, 2048)
                nc.sync.dma_start(hma, xm[0:128, qsl])
                nc.sync.dma_start(hmb[0:64, :], xm[128:192, qsl])
                hmA.append(hma)
                hmB.append(hmb)
            mask = cpool.tile([128, 128], f16)
            nc.sync.dma_start(mask, mask_d[:, :])
            maskv = mask.rearrange("p (i d) -> p i d", i=16)

            # PE pstate warmup: the Tensor engine reaches full clock only
            # after ~3us of continuous execution. Stream throwaway matmuls
            # on the query block while the moving-data DMA is in flight.
            wps = gpsum.tile([128, 2048], f32, tag="ps")
            for w in range(3):
                for cch in range(4):
                    msl = slice(cch * 512, (cch + 1) * 512)
                    nc.tensor.matmul(
                        wps[:, msl], hqA[:, 0:128], hqA[:, msl], start=True, stop=True
                    )

            # all tiles' scan outputs accumulate in SBUF; one DMA at the end
            iall = cpool.tile([128, 16 * 32], u16, tag="iall")
            vall = cpool.tile([128, 16 * 32], f16, tag="vall")

            for t in range(nt):
                tsl = ts(t, 128)
                S = spool.tile([128, N], f16, tag="s")
                for q in range(4):
                    ps = gpsum.tile([128, 2048], f32, tag="ps")
                    for cch in range(4):
                        msl = slice(cch * 512, (cch + 1) * 512)
                        nc.tensor.matmul(
                            ps[:, msl], hqA[:, tsl], hmA[q][:, msl],
                            start=True, stop=False,
                        )
                        nc.tensor.matmul(
                            ps[:, msl], hqB[:, tsl], hmB[q][:, msl],
                            start=False, stop=True,
                        )
                    # ACT drain + fp32->fp16 cast
                    nc.scalar.copy(S[:, ts(q, 2048)], ps)
                sviews = S.rearrange("p (i f) -> p i f", i=16)
                v3 = fpool.tile([128, NSLOT], f16, tag="v3")
                if t < nt - 1:
                    # knock out the self column (one cell per row, position
                    # i*512 + 8t + d with i = p%16, d = p//16), one op
                    sv = sviews[:, :, 8 * t : 8 * t + 8]
                    nc.vector.tensor_add(sv, sv, maskv)
                    # fp16 fold tree -> [128, 256] slot maxima
                    T = fpool.tile([128, 4096], f16, tag="T")
                    nc.vector.tensor_max(T, S[:, 0:4096], S[:, 4096:8192])
                    U = fpool.tile([128, 2048], f16, tag="U")
                    nc.vector.tensor_max(U, T[:, 0:2048], T[:, 2048:4096])
                    v1 = fpool.tile([128, 1024], f16, tag="v1")
                    nc.vector.tensor_max(v1, U[:, 0:1024], U[:, 1024:2048])
                    v2 = fpool.tile([128, 512], f16, tag="v2")
                    nc.vector.tensor_max(v2, v1[:, 0:512], v1[:, 512:1024])
                    nc.vector.tensor_max(v3, v2[:, 0:256], v2[:, 256:512])
                else:
                    # last tile: mask + first-level fold per quarter so the
                    # serial tail chain after the final drain is short
                    F = []
                    for q in range(4):
                        sv = sviews[:, 4 * q : 4 * q + 4, 8 * t : 8 * t + 8]
                        nc.vector.tensor_add(sv, sv, maskv[:, 4 * q : 4 * q + 4, :])
                        fq = fpool.tile([128, 1024], f16, tag=f"F{q}")
                        qs = 2048 * q
                        nc.vector.tensor_max(
                            fq, S[:, qs : qs + 1024], S[:, qs + 1024 : qs + 2048]
                        )
                        F.append(fq)
                    g01 = fpool.tile([128, 1024], f16, tag="g01")
                    nc.vector.tensor_max(g01, F[0], F[1])
                    g23 = fpool.tile([128, 1024], f16, tag="g23")
                    nc.vector.tensor_max(g23, F[2], F[3])
                    h = fpool.tile([128, 1024], f16, tag="h")
                    nc.vector.tensor_max(h, g01, g23)
                    v2 = fpool.tile([128, 512], f16, tag="v2")
                    nc.vector.tensor_max(v2, h[:, 0:512], h[:, 512:1024])
                    nc.vector.tensor_max(v3, v2[:, 0:256], v2[:, 256:512])
                # top-8 slots per 64-slot quarter
                for sq4 in range(4):
                    osl = slice(32 * t + 8 * sq4, 32 * t + 8 * sq4 + 8)
                    isl = slice(64 * sq4, 64 * (sq4 + 1))
                    nc.vector.max(out=vall[:, osl], in_=v3[:, isl])
                    nc.vector.max_index(iall[:, osl], vall[:, osl], v3[:, isl])
            nc.sync.dma_start(
                idx_out.rearrange("(t p) j -> p t j", t=16),
                iall.rearrange("p (t j) -> p t j", t=16),
            )
            nc.sync.dma_start(
                val_out.rearrange("(t p) j -> p t j", t=16),
                vall.rearrange("p (t j) -> p t j", t=16),
            )

    nc.compile()
    return nc


def _get_nc():
    if "nc" not in _cache:
        _cache["nc"] = _build_nc()
    return _cache["nc"]


def _host_prep(x):
    """Normalize along C in fp32 (reference semantics), cast fp16."""
    xs = np.ascontiguousarray(np.asarray(x, dtype=np.float32).reshape(B, C, N))
    nrm = np.sqrt((xs * xs).sum(axis=1, keepdims=True))
    xn = xs / np.maximum(nrm, 1e-12)  # [B, C, N] f32
    h16 = xn.astype(np.float16)
    return xn, h16


_PERM = None


def _perm():
    global _PERM
    if _PERM is None:
        c = np.arange(N)
        _PERM = np.empty(N, np.int64)
        _PERM[(c % 16) * 512 + c // 16] = c  # P(c) = (c%16)*512 + c//16
    return _PERM


def shard_inputs(h16):
    """h16: [B, C, N] f16 -> 8 per-core input maps."""
    perm = _perm()
    in_maps = []
    for c in range(NCORES):
        b, r = divmod(c, 4)
        s = r * RBLK
        rot = np.roll(h16[b], -s, axis=1) if s else h16[b]
        in_maps.append(
            {
                "xm": np.ascontiguousarray(rot[:, perm]),
                "xq": np.ascontiguousarray(rot[:, :RBLK]),
            }
        )
    return in_maps


def assemble(results, xn):
    """Exact rerank of the 1024 screened candidates per row.

    results: 8 dicts with 'idx8' [RBLK, 32] u16; cols [8q:8q+8] hold the
    top slot indices (in [0,64)) of slot-quarter q. Slot s covers the
    rotated columns 16*s+j and 16*(s+256)+j, j=0..15. Rank by f32
    distance, then refine the top 16 in f64; ties by smaller index ==
    jax top_k order.
    """
    nn = np.empty((B, N, 9), np.int32)
    qoff = np.repeat(np.arange(4, dtype=np.int64) * 64, 8)[None, :]  # [1, 32]
    wcols = np.arange(16, dtype=np.int64)[None, None, :]
    sq64 = [(xn[b].astype(np.float64) ** 2).sum(axis=0) for b in range(B)]
    xbT = [np.ascontiguousarray(xn[b].T) for b in range(B)]  # [N, C] f32
    xbT64 = [a.astype(np.float64) for a in xbT]
    for c in range(NCORES):
        b, r = divmod(c, 4)
        s = r * RBLK
        slots = results[c]["idx8"].astype(np.int64) + qoff  # [RBLK, 32]
        cand = np.concatenate(
            [
                slots[:, :, None] * 16 + wcols,
                (slots[:, :, None] + 256) * 16 + wcols,
            ],
            axis=1,
        )  # [RBLK, 64, 16] rotated cols
        cand = ((cand + s) % N).reshape(RBLK, 1024)
        rows = np.arange(s, s + RBLK)
        # f32 screen over all candidates
        qv = xbT[b][rows]  # [RBLK, C] f32
        vecs = xbT[b][cand]  # [RBLK, 1024, C] f32
        inner = np.matmul(vecs, qv[:, :, None])[:, :, 0]
        d32 = -2.0 * inner + (vecs**2).sum(-1)
        d32[cand == rows[:, None]] = np.inf
        part = np.argpartition(d32, 16, axis=1)[:, :16]
        cand16 = np.take_along_axis(cand, part, axis=1)  # [RBLK, 16]
        # f64 exact rerank of the survivors
        qv64 = xbT64[b][rows]
        vecs64 = xbT64[b][cand16]
        inner64 = np.matmul(vecs64, qv64[:, :, None])[:, :, 0]
        d = sq64[b][rows][:, None] + sq64[b][cand16] - 2.0 * inner64
        d[cand16 == rows[:, None]] = np.inf
        order = np.lexsort((cand16, d), axis=-1)[:, :8]
        nn[b, rows, 0] = rows
        nn[b, rows, 1:] = np.take_along_axis(cand16, order, axis=1)
    center = np.broadcast_to(np.arange(N, dtype=np.int32)[None, :, None], (B, N, 9))
    return np.ascontiguousarray(np.stack([nn, center], axis=0).astype(np.int32))


def kernel(x, _trace=False, **trace_kwargs):
    from concourse.bass_utils import run_bass_kernel_spmd

    nc = _get_nc()
    xn, h16 = _host_prep(x)
    in_maps = shard_inputs(h16)
    res = run_bass_kernel_spmd(
        nc, in_maps, core_ids=list(range(NCORES)), trace=_trace, **trace_kwargs
    )
    _cache["last_results"] = res
    return assemble(res.results, xn)


# revision 12
# speedup vs baseline: 1.0136x; 1.0136x over previous
"""KNN graph kernel (DenseDilatedKnnGraph) for Trainium2, 8 NeuronCores.

Problem: x [2, 192, 8192, 1] fp32 -> edge_index [2, 2, 8192, 9] int32.
reference: L2-normalize x along C, pairwise sq-dists over N, top-9 (k=9,
dilation=1) nearest neighbors (indices), stacked with center indices.

Strategy (candidate-screen + exact host rerank):
  For normalized points, ranking by -dist == ranking by cosine Xn^T Xn.
  The device computes an fp16 SCREENING Gram (2 K=128 matmul passes per
  512-col chunk; B channels zero-padded to K=128 so the PE tile config
  never changes -- a K=64/K=128 alternation drops the PE to its mid
  pstate) and reduces each query row's 8192 columns to a 256-slot
  column-max array via a DVE fp16 tensor_max fold tree (fp16
  tensor_tensor runs at 2x_1p; MAX8/FIND_INDEX8 have no fast mode, so
  their scan area must be small). Top-8 slots of each 64-slot quarter
  -> 32 slots x 32 cols = 1024 candidate columns per row; the host
  re-ranks candidates exactly (f32 screen + f64 top-16 refine) against
  the fp32 normalized points, reproducing the reference top-8.

  Slot geometry: the host PERMUTES moving columns P(c) = (c%16)*512 +
  c//16, so slot s (= permuted col mod 256) covers the 16-contiguous-
  col groups {16s..16s+15, 16(s+256)..16(s+256)+15}. The data is
  diagonally correlated (neighbors cluster at col ~ row), so contiguous
  groups absorb clusters of near neighbors into one slot instead of
  crowding many distinct slots. Top-8-per-quarter is safe: a true top-8
  item's slot can have at most 7 slots with larger max anywhere (each
  must contain a better item), so only fp16 ties at the exact 8/9
  quarter boundary can drop it -- requiring all 7 above-slots in one
  quarter, P ~ (1/4)^7 per row, ~0 expected over 16k rows.

  The self column (cos ~ 1) is knocked out with -20 via one constant
  [128,16,8] mask (its permuted position is row-dependent) before
  folding; the host prepends the self index (reference rank-1 neighbor
  is always self).

Sharding: 8 cores = 2 batches x 4 query-row-blocks of 2048. Each core
gets the full batch slice with columns ROTATED so its query block sits
at rotated columns 0..2047. Host maps indices back via the offset.

Per row-tile (128 rows x 8192 cols) engine budget, measured rates:
  PE   32 matmuls at ~248ns cadence             = 7.9us   <- bound
  ACT  4 drains [128,2048] PSUM->SBUF fp16      = 7.9us   <- bound
  DVE  folds 4.7 + mask 0.2 + scans 1.8         = 6.7us
Startup ~10us (6us NEFF preamble + input DMA, overlapped with PE
pstate warmup), tail ~5us (last tile's fold+scan chain).
"""

import numpy as np

B = 2
C = 192
N = 8192
NCORES = 8
RBLK = N // 4  # 2048 query rows per core
NT = RBLK // 128  # 16 row tiles per core
NSLOT = 256
NEG = -20.0

_cache = {}


def _self_mask():
    """m[p, i, d] = NEG iff the self column of query row p lands at
    permuted column i*512 + 8t + d (t enters via the AP column offset;
    the mask data is t-independent)."""
    m = np.zeros((128, 16, 8), np.float16)
    for p in range(128):
        m[p, p % 16, p // 16] = np.float16(NEG)
    return m.reshape(128, 128)


def _build_nc(nt=NT):
    import concourse.bacc as bacc
    import concourse.mybir as mybir
    from concourse.bass import ts
    from concourse.tile import TileContext

    f32 = mybir.dt.float32
    f16 = mybir.dt.float16
    u16 = mybir.dt.uint16

    nc = bacc.Bacc("TRN2")

    # moving columns: rotated + permuted; query columns: rotated only
    xm = nc.dram_tensor("xm", [C, N], f16, kind="ExternalInput")
    xq = nc.dram_tensor("xq", [C, RBLK], f16, kind="ExternalInput")
    idx_out = nc.dram_tensor("idx8", [RBLK, 32], u16, kind="ExternalOutput")
    val_out = nc.dram_tensor("val8", [RBLK, 32], f16, kind="ExternalOutput")

    mask_d = nc.inline_tensor(_self_mask(), name="selfmask")

    with TileContext(nc) as tc:
        with (
            tc.tile_pool(name="consts", bufs=1) as cpool,
            tc.tile_pool(name="xpool", bufs=1) as xpool,
            tc.tile_pool(name="spool", bufs=2) as spool,
            tc.tile_pool(name="fpool", bufs=3) as fpool,
            tc.tile_pool(name="vpool", bufs=3) as vpool,
            tc.tile_pool(name="gpsum", bufs=2, space="PSUM") as gpsum,
        ):
            # fp16 normalized points (host-prepared). A: channels 0..127;
            # B: channels 128..191 in rows 0..63, rows 64..127 zeroed.
            # DMA issues are ~620ns each, serialized on the Sync engine:
            # order = query block first (unblocks the PE warmup), then
            # moving blocks (first real tile depends only on block 0),
            # mask last.
            hqA = xpool.tile([128, RBLK], f16)
            hqB = xpool.tile([128, RBLK], f16)
            nc.gpsimd.memset(hqB[64:128, :], 0.0)
            nc.sync.dma_start(hqA, xq[0:128, :])
            nc.sync.dma_start(hqB[0:64, :], xq[128:192, :])
            hmA, hmB = [], []
            for q in range(4):
                hma = xpool.tile([128, 2048], f16, tag=f"hmA{q}")
                hmb = xpool.tile([128, 2048], f16, tag=f"hmB{q}")
                nc.gpsimd.memset(hmb[64:128, :], 0.0)
                qsl = ts(q, 2048)
                nc.sync.dma_start(hma, xm[0:128, qsl])
                nc.sync.dma_start(hmb[0:64, :], xm[128:192, qsl])
                hmA.append(hma)
                hmB.append(hmb)
            mask = cpool.tile([128, 128], f16)
            nc.sync.dma_start(mask, mask_d[:, :])
            maskv = mask.rearrange("p (i d) -> p i d", i=16)

            # PE pstate warmup: the Tensor engine reaches full clock only
            # after ~3us of continuous execution. Stream throwaway matmuls
            # on the query block while the moving-data DMA is in flight.
            wps = gpsum.tile([128, 2048], f32, tag="ps")
            for w in range(6):
                for cch in range(4):
                    msl = slice(cch * 512, (cch + 1) * 512)
                    nc.tensor.matmul(
                        wps[:, msl], hqA[:, 0:128], hqA[:, msl], start=True, stop=True
                    )

            for t in range(nt):
                tsl = ts(t, 128)
                S = spool.tile([128, N], f16, tag="s")
                for q in range(4):
                    ps = gpsum.tile([128, 2048], f32, tag="ps")
                    for cch in range(4):
                        msl = slice(cch * 512, (cch + 1) * 512)
                        nc.tensor.matmul(
                            ps[:, msl], hqA[:, tsl], hmA[q][:, msl],
                            start=True, stop=False,
                        )
                        nc.tensor.matmul(
                            ps[:, msl], hqB[:, tsl], hmB[q][:, msl],
                            start=False, stop=True,
                        )
                    # ACT drain + fp32->fp16 cast
                    nc.scalar.copy(S[:, ts(q, 2048)], ps)
                # knock out the self column (one cell per row, position
                # i*512 + 8t + d with i = p%16, d = p//16)
                sv = S.rearrange("p (i f) -> p i f", i=16)[:, :, 8 * t : 8 * t + 8]
                nc.vector.tensor_add(sv, sv, maskv)
                # fp16 fold tree -> [128, 256] slot maxima
                T = fpool.tile([128, 4096], f16, tag="T")
                nc.vector.tensor_max(T, S[:, 0:4096], S[:, 4096:8192])
                U = fpool.tile([128, 2048], f16, tag="U")
                nc.vector.tensor_max(U, T[:, 0:2048], T[:, 2048:4096])
                v1 = fpool.tile([128, 1024], f16, tag="v1")
                nc.vector.tensor_max(v1, U[:, 0:1024], U[:, 1024:2048])
                v2 = fpool.tile([128, 512], f16, tag="v2")
                nc.vector.tensor_max(v2, v1[:, 0:512], v1[:, 512:1024])
                v3 = fpool.tile([128, NSLOT], f16, tag="v3")
                nc.vector.tensor_max(v3, v2[:, 0:256], v2[:, 256:512])
                # top-8 slots per 64-slot quarter
                v32 = vpool.tile([128, 32], f16)
                i32 = vpool.tile([128, 32], u16)
                for sq4 in range(4):
                    osl = slice(8 * sq4, 8 * sq4 + 8)
                    isl = slice(64 * sq4, 64 * (sq4 + 1))
                    nc.vector.max(out=v32[:, osl], in_=v3[:, isl])
                    nc.vector.max_index(i32[:, osl], v32[:, osl], v3[:, isl])
                nc.sync.dma_start(idx_out[tsl, :], i32)
                nc.sync.dma_start(val_out[tsl, :], v32)

    nc.compile()
    return nc


def _get_nc():
    if "nc" not in _cache:
        _cache["nc"] = _build_nc()
    return _cache["nc"]


def _host_prep(x):
    """Normalize along C in fp32 (reference semantics), cast fp16."""
    xs = np.ascontiguousarray(np.asarray(x, dtype=np.float32).reshape(B, C, N))
    nrm = np.sqrt((xs * xs).sum(axis=1, keepdims=True))
    xn = xs / np.maximum(nrm, 1e-12)  # [B, C, N] f32
    h16 = xn.astype(np.float16)
    return xn, h16


_PERM = None


def _perm():
    global _PERM
    if _PERM is None:
        c = np.arange(N)
        _PERM = np.empty(N, np.int64)
        _PERM[(c % 16) * 512 + c // 16] = c  # P(c) = (c%16)*512 + c//16
    return _PERM


def shard_inputs(h16):
    """h16: [B, C, N] f16 -> 8 per-core input maps."""
    perm = _perm()
    in_maps = []
    for c in range(NCORES):
        b, r = divmod(c, 4)
        s = r * RBLK
        rot = np.roll(h16[b], -s, axis=1) if s else h16[b]
        in_maps.append(
            {
                "xm": np.ascontiguousarray(rot[:, perm]),
                "xq": np.ascontiguousarray(rot[:, :RBLK]),
            }
        )
    return in_maps


def assemble(results, xn):
    """Exact rerank of the 1024 screened candidates per row.

    results: 8 dicts with 'idx8' [RBLK, 32] u16; cols [8q:8q+8] hold the
    top slot indices (in [0,64)) of slot-quarter q. Slot s covers the
    rotated columns 16*s+j and 16*(s+256)+j, j=0..15. Rank by f32
    distance, then refine the top 16 in f64; ties by smaller index ==
    jax top_k order.
    """
    nn = np.empty((B, N, 9), np.int32)
    qoff = np.repeat(np.arange(4, dtype=np.int64) * 64, 8)[None, :]  # [1, 32]
    wcols = np.arange(16, dtype=np.int64)[None, None, :]
    sq64 = [(xn[b].astype(np.float64) ** 2).sum(axis=0) for b in range(B)]
    xbT = [np.ascontiguousarray(xn[b].T) for b in range(B)]  # [N, C] f32
    xbT64 = [a.astype(np.float64) for a in xbT]
    for c in range(NCORES):
        b, r = divmod(c, 4)
        s = r * RBLK
        slots = results[c]["idx8"].astype(np.int64) + qoff  # [RBLK, 32]
        cand = np.concatenate(
            [
                slots[:, :, None] * 16 + wcols,
                (slots[:, :, None] + 256) * 16 + wcols,
            ],
            axis=1,
        )  # [RBLK, 64, 16] rotated cols
        cand = ((cand + s) % N).reshape(RBLK, 1024)
        rows = np.arange(s, s + RBLK)
        # f32 screen over all candidates
        qv = xbT[b][rows]  # [RBLK, C] f32
        vecs = xbT[b][cand]  # [RBLK, 1024, C] f32
        inner = np.matmul(vecs, qv[:, :, None])[:, :, 0]
        d32 = -2.0 * inner + (vecs**2).sum(-1)
        d32[cand == rows[:, None]] = np.inf
        part = np.argpartition(d32, 16, axis=1)[:, :16]
        cand16 = np.take_along_axis(cand, part, axis=1)  # [RBLK, 16]
        # f64 exact rerank of the survivors
        qv64 = xbT64[b][rows]
        vecs64 = xbT64[b][cand16]
        inner64 = np.matmul(vecs64, qv64[:, :, None])[:, :, 0]
        d = sq64[b][rows][:, None] + sq64[b][cand16] - 2.0 * inner64
        d[cand16 == rows[:, None]] = np.inf
        order = np.lexsort((cand16, d), axis=-1)[:, :8]
        nn[b, rows, 0] = rows
        nn[b, rows, 1:] = np.take_along_axis(cand16, order, axis=1)
    center = np.broadcast_to(np.arange(N, dtype=np.int32)[None, :, None], (B, N, 9))
    return np.ascontiguousarray(np.stack([nn, center], axis=0).astype(np.int32))


def kernel(x, _trace=False, **trace_kwargs):
    from concourse.bass_utils import run_bass_kernel_spmd

    nc = _get_nc()
    xn, h16 = _host_prep(x)
    in_maps = shard_inputs(h16)
    res = run_bass_kernel_spmd(
        nc, in_maps, core_ids=list(range(NCORES)), trace=_trace, **trace_kwargs
    )
    _cache["last_results"] = res
    return assemble(res.results, xn)
